# revision 2
# baseline (speedup 1.0000x reference)
"""Dense MoE (BasicMoE) Trainium2 Bass kernel — bf16, pipelined v2.

Problem (hardcoded): x [4, 2048, 1024] f32, gate_w [1024, 8], gate_b [8],
expert_w [8, 1024, 1024], expert_b [8, 1024].

Sharding: tokens split across 8 cores (data parallel), weights replicated.

The expert phase already runs at ~97% of the bf16 roofline; this version
attacks the ~16us of head/tail overhead around it. Trace-derived facts:
the NEFF preamble keeps every sequencer busy until ~6us; each DMA trigger
costs ~600ns of *serialized* sequencer time (DIRECT2D), so trigger counts
and placement — not just bytes — decide when operands land.

  1. PE warmup on a memzero'd tile (no iota/DMA dependency): the PE
     pstate ramps during the x DMA window, so the gate runs at full
     clock instead of half.
  2. gate_w is host-packed to [128, KT*E] so it loads with ONE trigger;
     x has the gpsimd queues to itself (16 triggers, half-token-major);
     expert0's W is all-sync in 2-chunk halves right behind it.
  3. Evictions are one fused DVE scalar_tensor_tensor (acc=ps*ews+acc).
  4. Output DMA triggers alternate sync/gpsimd so the final triggers
     drain in parallel instead of queueing 600ns each on sync.
"""

import os
from contextlib import ExitStack

import numpy as np

import concourse.tile as tile
from concourse import bacc, mybir
from concourse.bass_utils import run_bass_kernel_spmd
from concourse.masks import make_identity

B, S, H, E = 4, 2048, 1024, 8
T = B * S
N_CORES = 8
TL = T // N_CORES          # tokens per core = 1024
P = 128                    # SBUF partitions
KT = H // P                # 8 contraction tiles
MT = TL // P               # 8 token tiles per core
DH = 512                   # matmul moving free-dim (fp32 PSUM bank)
ND = H // DH               # 2 d-halves
XC = 1                     # x DMA chunks: 8 triggers, transfers parallelize
NWARM = 16                 # PE pstate warmup matmuls

F32 = mybir.dt.float32
F32R = mybir.dt.float32r
BF16 = mybir.dt.bfloat16

_CACHE = {}
LAST_RESULT = None


def _r(ap):
    """Bitcast an f32 AP to float32r (same bits; PE rounds internally)."""
    return ap.bitcast(F32R)


def _build_moe_nc(with_bias: bool):
    nc = bacc.Bacc(
        "TRN2",
        target_bir_lowering=False,
        debug=False,
        enable_asserts=False,
        num_devices=N_CORES,
    )

    x_shT = nc.dram_tensor("x_shT", [H, TL], BF16, kind="ExternalInput").ap()
    # host-packed [p][k][e] so it is a single straight DMA
    gate_w = nc.dram_tensor("gate_w", [P, KT * E], BF16, kind="ExternalInput").ap()
    gate_b = nc.dram_tensor("gate_b", [E], F32, kind="ExternalInput").ap()
    expert_w = nc.dram_tensor("expert_w", [E, H, H], BF16, kind="ExternalInput").ap()
    expert_b = nc.dram_tensor("expert_b", [E, H], F32, kind="ExternalInput").ap()
    out_sh = nc.dram_tensor("out_sh", [TL, H], F32, kind="ExternalOutput").ap()

    with tile.TileContext(nc) as tc, ExitStack() as ctx:
        const = ctx.enter_context(tc.tile_pool(name="const", bufs=1))
        wpool = ctx.enter_context(tc.tile_pool(name="wpool", bufs=2))
        accp = ctx.enter_context(tc.tile_pool(name="accp", bufs=1))
        tmp = ctx.enter_context(tc.tile_pool(name="tmp", bufs=6))
        # main psum pool FIRST: its banks must not overlap the gate pool's,
        # else Tile's released-zone dep would stall expert 0's first matmul
        # group behind the whole gate phase.
        psum = ctx.enter_context(tc.tile_pool(name="psum", bufs=6, space="PSUM"))
        psum_s = tc.alloc_tile_pool(name="psum_s", bufs=1, space="PSUM")

        # ---- PE pstate warmup -------------------------------------------
        # Real (identity) data, no DMA dependency. An all-zero warmup tile
        # does NOT ramp the clock (no switching activity): measured 258ns
        # steady-state matmuls for the whole kernel instead of 216ns.
        identw = const.tile([P, P], F32, name="identw")
        make_identity(nc, identw)
        for wi in range(NWARM):
            pw = psum.tile([P, DH], F32, tag="ps")
            nc.tensor.matmul(
                pw[:, 0:P], lhsT=identw, rhs=identw, start=True, stop=True
            )

        ident_bf = const.tile([E, E], BF16)
        make_identity(nc, ident_bf)
        if with_bias:
            ident = const.tile([P, P], F32)
            make_identity(nc, ident)

        # ---- loads ------------------------------------------------------
        # sync engine: gate_w (1 trigger), gate_b, then expert0's chunks.
        gw = const.tile([P, KT * E], BF16)
        nc.sync.dma_start(gw, gate_w)
        gb8 = const.tile([E, 1], F32)
        nc.sync.dma_start(gb8, gate_b[:, None])
        if with_bias:
            eb = const.tile([E, H], F32R)
            nc.sync.dma_start(eb, _r(expert_b))

        # xT: h on partitions, t on free — straight (contiguous) DMA from
        # the host-transposed shard. GpSimd SWDGE queues only (the sync
        # sequencer's ~600ns/trigger budget is reserved for W), half-token
        # chunks in half-major order so the first gate chain lands first.
        xT = [const.tile([P, TL], BF16, name=f"xT{k}") for k in range(KT)]
        xcw = TL // XC
        for c in range(XC):
            csl = slice(c * xcw, (c + 1) * xcw)
            for k in range(KT):
                nc.gpsimd.dma_start(xT[k][:, csl], x_shT[k * P : (k + 1) * P, csl])

        # ---- gate -------------------------------------------------------
        ewT_raw = const.tile([E, TL], BF16)   # exp(logits).T (unnormalized)
        ews = const.tile([P, MT, E], F32)     # per-token gate weight / S
        ewsT = None
        if with_bias:
            ewsT = const.tile([E, TL], F32R, name="ewsT")

        for c in range(ND):
            csl = slice(c * DH, (c + 1) * DH)
            pgT = psum_s.tile([E, DH], F32, tag="sm", bufs=2)
            for k in range(KT):
                nc.tensor.matmul(
                    pgT,
                    lhsT=gw[:, k * E : (k + 1) * E],
                    rhs=xT[k][:, csl],
                    start=(k == 0),
                    stop=(k == KT - 1),
                )
            # ewT = exp(logitsT + gate_b); gate_b is per-partition here
            nc.scalar.activation(
                ewT_raw[:, csl], pgT, mybir.ActivationFunctionType.Exp, bias=gb8
            )

        for m in range(MT):
            msl = slice(m * P, (m + 1) * P)
            # ew[t, e] for this token tile via PE transpose
            ptw = psum_s.tile([P, E], BF16, tag="sm", bufs=2)
            nc.tensor.transpose(ptw, ewT_raw[:, msl], ident_bf)
            ssum = tmp.tile([P, 1], F32, tag="ssum")
            nc.vector.reduce_sum(ssum, ptw, axis=mybir.AxisListType.X)
            inv = tmp.tile([P, 1], F32, tag="inv")
            nc.vector.reciprocal(inv, ssum)
            nc.vector.tensor_scalar_mul(ews[:, m, :], ptw, inv)
            if with_bias:
                # back-transpose the normalized weights for the bias matmul
                ptb = psum_s.tile([E, P], F32, tag="sm", bufs=2)
                nc.tensor.transpose(ptb, ews[:, m, :], ident)
                nc.vector.tensor_copy(ewsT[:, msl], _r(ptb))

        # gate done; its banks are no longer needed
        psum_s.release()

        # ---- bias seed: acc = ews @ expert_b (skipped for zero bias) ---
        acc = [accp.tile([P, H], F32, name=f"acc{m}") for m in range(MT)]
        if with_bias:
            for m in range(MT):
                msl = slice(m * P, (m + 1) * P)
                for n in range(ND):
                    nsl = slice(n * DH, (n + 1) * DH)
                    pb = psum.tile([P, DH], F32, tag="ps")
                    nc.tensor.matmul(
                        pb, lhsT=ewsT[:, msl], rhs=eb[:, nsl], start=True, stop=True
                    )
                    nc.vector.tensor_copy(acc[m][:, nsl], pb)

        # ---- experts ----------------------------------------------------
        for e in range(E):
            wsb = wpool.tile([P, KT, H], BF16, tag="w")
            # expert0 in 2-chunk halves (queue parallelism for the transfer
            # the PE is waiting on); steady state one sync chunk per k.
            ewc = 2 if e == 0 else 1
            wcw = H // ewc
            for c in range(ewc):
                for k in range(KT):
                    csl = slice(c * wcw, (c + 1) * wcw)
                    nc.sync.dma_start(
                        wsb[:, k, csl],
                        expert_w[e, k * P : (k + 1) * P, csl],
                    )
            last = e == E - 1
            for n in range(ND):
                nsl = slice(n * DH, (n + 1) * DH)
                for m in range(MT):
                    msl = slice(m * P, (m + 1) * P)
                    ps = psum.tile([P, DH], F32, tag="ps")
                    for k in range(KT):
                        nc.tensor.matmul(
                            ps,
                            lhsT=xT[k][:, msl],
                            rhs=wsb[:, k, nsl],
                            start=(k == 0),
                            stop=(k == KT - 1),
                        )
                    # fused evict: acc = ps*ews + acc in one DVE op. With no
                    # bias seed, expert 0 writes acc directly (ACT/DVE).
                    if e == 0 and not with_bias:
                        if (m + n) % 2 == 0:
                            nc.scalar.mul(acc[m][:, nsl], ps, ews[:, m, e : e + 1])
                        else:
                            nc.vector.tensor_scalar_mul(
                                acc[m][:, nsl], ps, ews[:, m, e : e + 1]
                            )
                    else:
                        nc.vector.scalar_tensor_tensor(
                            acc[m][:, nsl],
                            ps,
                            ews[:, m, e : e + 1],
                            acc[m][:, nsl],
                            op0=mybir.AluOpType.mult,
                            op1=mybir.AluOpType.add,
                        )
                    if last:
                        if m == MT - 1:
                            # final groups: halves on both engines in parallel
                            hw2 = DH // 2
                            for ci, oeng in ((0, nc.sync), (1, nc.gpsimd)):
                                osl = slice(
                                    n * DH + ci * hw2, n * DH + (ci + 1) * hw2
                                )
                                oeng.dma_start(
                                    out_sh[m * P : (m + 1) * P, osl],
                                    acc[m][:, osl],
                                )
                        else:
                            # one trigger per group, alternating engines so
                            # the tail triggers drain in parallel
                            oeng = nc.sync if (m + n) % 2 == 0 else nc.gpsimd
                            oeng.dma_start(
                                out_sh[m * P : (m + 1) * P, nsl],
                                acc[m][:, nsl],
                            )

    nc.compile()
    return nc


def kernel(**inputs) -> np.ndarray:
    global LAST_RESULT
    import ml_dtypes

    bf16 = ml_dtypes.bfloat16
    x = np.asarray(inputs["x"], dtype=np.float32).reshape(T, H)
    gw = np.asarray(inputs["gate_w"], dtype=np.float32).astype(bf16)
    # pack [H, E] -> [p][k][e] so the device load is one straight DMA
    gwr = np.ascontiguousarray(
        gw.reshape(KT, P, E).transpose(1, 0, 2).reshape(P, KT * E)
    )
    gb = np.ascontiguousarray(np.asarray(inputs["gate_b"], dtype=np.float32))
    ew = np.ascontiguousarray(np.asarray(inputs["expert_w"], dtype=np.float32).astype(bf16))
    eb = np.ascontiguousarray(np.asarray(inputs["expert_b"], dtype=np.float32))

    with_bias = bool(np.any(eb))
    key = ("nc", with_bias)
    if key not in _CACHE:
        _CACHE[key] = _build_moe_nc(with_bias)
    nc = _CACHE[key]

    in_maps = [
        {
            "x_shT": np.ascontiguousarray(x[c * TL : (c + 1) * TL].T.astype(bf16)),
            "gate_w": gwr,
            "gate_b": gb,
            "expert_w": ew,
            "expert_b": eb,
        }
        for c in range(N_CORES)
    ]
    res = run_bass_kernel_spmd(
        nc,
        in_maps,
        core_ids=list(range(N_CORES)),
        trace=bool(int(os.environ.get("MOE_TRACE", "0"))),
    )
    LAST_RESULT = res
    out = np.concatenate([res.results[c]["out_sh"] for c in range(N_CORES)], axis=0)
    return out.reshape(B, S, H)
